# revision 43
# baseline (speedup 1.0000x reference)
"""Trainium2 Bass kernel for EfficientViT-style attention block.

Reference computation (per batch element b of 16):
    x: [256, 1024]  (C=256 channels, N=32*32 spatial)
    q = (sq*wq) @ x + bq        -> [128, N]  (8 heads x 16 key dims)
    k = (sk*wk) @ x + bk        -> [128, N]
    v = (sv*wv) @ x + bv        -> [256, N]  (8 heads x 32 v dims)
    per head: attn = softmax(q_h^T k_h, axis=-1); o_h = v_h @ attn^T
    out = (sp*wp) @ relu(concat o_h) + bp

Sharding: data-parallel over batch: 8 cores x 2 batch elements. No
collectives; full inputs sharded host-side, outputs concatenated.

Kernel strategy per core (all matmuls bf16 inputs, fp32 PSUM accumulate):
- scale factors folded into weights host-side; Q/K output channels
  pre-permuted into "padded head groups": group g holds heads 4g..4g+3 at
  32-partition stride so per-head score matmuls are tile_position-packable.
- scores computed transposed, ST[m, n] = k_h^T q_h, 4 heads row-packed into
  four single-bank PSUM tiles. exp is split 50/50 ACROSS ENGINES per m-tile:
  heads 0/2 exact exp on ScalarE (PSUM->SBUF bf16); heads 1/3 fast-exp on
  VectorE via a bit trick: int16(round(S*128*log2(e) + 127*128 - 7.3))
  reinterpreted as bf16 is exp(S) to ~2% (linear mantissa interp). Numerator
  and denominator use the same approx values, so softmax weights still sum
  to 1. Each engine releases its own score banks independently, which keeps
  the score->exp->score chain short; heads 0/1 use dedicated banks (their
  release is always prompt), heads 2/3 plus the projection pieces share a
  4-bank rotation that absorbs consumer jitter.
- lag-0 software pipeline: each iteration (b, head-group, n-half) consumes
  its own exp output. AV/denominator waves trail the exp by two m-tiles so
  a queued wave never blocks the next scores in the in-order PE queue
  (waves need the LATEST exp of their m-tile, scores only the earliest).
- AV: o_h += vT_h(m-tile)^T @ E(m-tile), 4 heads col-group-packed waves,
  then the denominator wave: ones[128,32] stationary against E -> denom
  replicated across each head's 32 partitions (reduction + broadcast in one
  matmul). V bias folded into vt at the PSUM->SBUF copy (tensor_tensor add
  with a broadcast constant) so o/den = attn_out + bv exactly.
- normalize: relu-before-divide (valid since den>0): r = max(av,0)*recip(den)
  in two DVE ops; q/k/v/y PSUM->SBUF bias-copies ride on ScalarE
  (activation Identity + per-partition bias) to balance engine load;
  bf16 output, upcast on host.
"""

import numpy as np
import ml_dtypes

B, C, H, W = 16, 256, 32, 32
N = H * W            # 1024
NH, KD, DV = 8, 16, 32
NB = 2               # batch elements per core
NCORES = 8
P = 128
NT = 512             # n-tile (psum bank)

# per-mt count of 512-col PSUM banks (of the sb score tile) handled by the
# DVE fast-exp instead of ScalarE exact exp. sum/32 = DVE share of exp work.
NDVE = (2, 1, 2, 1, 2, 1, 2, 1)
FE_SCALE = float(128.0 / np.log(2.0))          # 184.6644
FE_OFF = float(127.0 * 128.0 - 7.3)            # centers the interp error

BF16 = ml_dtypes.bfloat16

_CACHE = {}


def _build_nc():
    import concourse.tile as tile
    from concourse import bacc, mybir

    f32 = mybir.dt.float32
    bf16 = mybir.dt.bfloat16
    i16 = mybir.dt.int16
    Alu = mybir.AluOpType
    Act = mybir.ActivationFunctionType

    # Bacc (not raw Bass): its finalize() runs generate_event_semaphores,
    # which splits multi-sem waits — TRN2 instructions take at most one.
    nc = bacc.Bacc()

    xb = nc.declare_dram_parameter("xb", [NB, C, N], bf16, isOutput=False)
    # all weights in one tensor, partition-major: [p, ct, {wq|wk|wv|wp} x 256]
    wall = nc.declare_dram_parameter("wall", [P, 2, 4 * 256], bf16,
                                     isOutput=False)
    # per-partition bias vectors: [partition, group, {q,k,-,p}]
    biases = nc.declare_dram_parameter("biases", [P, 2, 4], f32, isOutput=False)
    # v-bias broadcast across partitions: [p, dh]
    bvb = nc.declare_dram_parameter("bvb", [P, 2 * P], f32, isOutput=False)
    out = nc.declare_dram_parameter("out", [NB, C, N], bf16, isOutput=True)

    with tile.TileContext(nc) as tc:
        with (
            tc.tile_pool(name="consts", bufs=1) as consts,
            tc.tile_pool(name="xp", bufs=2) as xp,
            tc.tile_pool(name="qk", bufs=2) as qk,
            tc.tile_pool(name="vtp", bufs=2) as vtp,
            tc.tile_pool(name="ep", bufs=3) as ep,
            tc.tile_pool(name="rp", bufs=2) as rp,
            tc.tile_pool(name="yp", bufs=4) as yp,
            tc.tile_pool(name="ps_s", bufs=1, space="PSUM") as ps_s,
            tc.tile_pool(name="ps_av", bufs=1, space="PSUM") as ps_av,
            tc.tile_pool(name="ps_den", bufs=1, space="PSUM") as ps_den,
        ):
            rot_ctr = [0]

            def rot_tile(name):
                t = ps_s.tile([P, NT], f32, tag=f"r{rot_ctr[0] % 4}",
                              name=name)
                rot_ctr[0] += 1
                return t

            # --- memsets + PE warmup first (no DMA deps): HAM ramps to
            # full clock while the input DMAs are in flight ---
            ones_sb = consts.tile([P, DV], bf16, tag="ones")
            nc.vector.memset(ones_sb[:], 1.0)
            warm_rhs = consts.tile([P, NT], bf16, tag="warm_rhs")
            nc.vector.memset(warm_rhs[:], 0.0)
            for wi in range(6):
                wps = rot_tile(f"warm{wi}")
                nc.tensor.matmul(wps[0:DV, 0:256], lhsT=ones_sb[:],
                                 rhs=warm_rhs[:, 0:256], start=True, stop=True)

            # --- constants into SBUF: x for b=0 first (gates the first
            # projections), then weights (q|k halves first), then biases ---
            x0_sb = xp.tile([P, 2, N], bf16, tag="x")
            for ct in range(2):
                nc.sync.dma_start(out=x0_sb[:, ct, :],
                                  in_=xb[0, ct * P:(ct + 1) * P, :])
            w_sb = consts.tile([P, 2, 4 * 256], bf16, tag="w")
            nc.gpsimd.dma_start(out=w_sb[:, :, 0:512], in_=wall[:, :, 0:512])
            nc.gpsimd.dma_start(out=w_sb[:, :, 512:1024],
                                in_=wall[:, :, 512:1024])
            bias_sb = consts.tile([P, 2, 4], f32, tag="bias")
            nc.gpsimd.dma_start(out=bias_sb[:], in_=biases[:])
            bvb_sb = consts.tile([P, 2 * P], f32, tag="bvb")
            nc.gpsimd.dma_start(out=bvb_sb[:], in_=bvb[:])
            # "touch" ops: bring the DVE/ACT vector clocks past the constant
            # DMAs so downstream TensorScalar instructions (1-wait-limited in
            # walrus codegen) only ever wait on the PE semaphore. The ACT
            # touch also pre-loads the exp table set.
            scratch = consts.tile([P, 2], f32, tag="scratch")
            nc.vector.tensor_copy(out=scratch[:, 0:1], in_=bias_sb[:, 0, 0:1])
            nc.scalar.activation(out=scratch[:, 1:2], in_=bias_sb[:, 0, 1:2],
                                 func=Act.Exp)

            def bias_ap(kind, g):
                i = {"q": 0, "k": 1, "v": 2, "p": 3}[kind]
                return bias_sb[:, g, i:i + 1]


            # ---------- per-b building blocks ----------
            qkv = {}    # b -> dict(q=, k=, vt=)
            r_tiles = {}  # b -> r_sb

            def qkv_piece(b, kind, g, nt_or_mt, boot_pool=None):
                """One projection piece: kind in {q, k, vt}. boot_pool lets
                the first pieces borrow the (still idle) av/den psum banks so
                their matmuls all issue at once instead of rotating through
                the two proj banks — pulls the first exp several us earlier."""
                x_sb = qkv[b]["x"]
                if kind in ("q", "k"):
                    woff = 0 if kind == "q" else 256
                    dst = qkv[b][kind]
                    nt = nt_or_mt
                    ps = rot_tile(f"pp_{b}{kind}{g}{nt}")
                    for ct in range(2):
                        nc.tensor.matmul(
                            ps[:],
                            lhsT=w_sb[:, ct, woff + P * g:woff + P * (g + 1)],
                            rhs=x_sb[:, ct, nt * NT:(nt + 1) * NT],
                            start=(ct == 0), stop=(ct == 1))
                    # PSUM->SBUF bias-copies ride on ScalarE (the DVE is
                    # as busy once it carries half the exp work); the boot
                    # q-piece uses the then-idle DVE so the three boot
                    # copies overlap
                    if b == 0 and g == 0 and nt_or_mt == 0 and kind == "k":
                        nc.vector.tensor_scalar_add(
                            out=dst[:, g, nt * NT:(nt + 1) * NT],
                            in0=ps[:], scalar1=bias_ap(kind, g))
                    else:
                        nc.scalar.activation(
                            out=dst[:, g, nt * NT:(nt + 1) * NT],
                            in_=ps[:], func=Act.Identity,
                            bias=bias_ap(kind, g))
                else:
                    mt = nt_or_mt
                    ps = rot_tile(f"pv_{b}{mt}")
                    for ct in range(2):
                        nc.tensor.matmul(
                            ps[:, 0:256],
                            lhsT=x_sb[:, ct, mt * P:(mt + 1) * P],
                            rhs=w_sb[:, ct, 512:768],
                            start=(ct == 0), stop=(ct == 1))
                    # fold bv in during the PSUM->SBUF copy (same DVE cost)
                    nc.vector.tensor_tensor(
                        out=qkv[b]["vt"][:, mt, :], in0=ps[:, 0:256],
                        in1=bvb_sb[:], op=Alu.add)

            def emit_qkv_head(b):
                """Allocate b's tiles + the minimum pieces for its first
                scores: k(g0, both nt) and q(g0, nt0). Returns the deferred
                piece closures to spread into the pipeline."""
                if b == 0:
                    x_sb = x0_sb
                else:
                    # issue later-batch x loads from the (busy) scalar queue:
                    # the idle sync engine would fire them during boot and
                    # steal HBM bandwidth from the critical weight DMA
                    x_sb = xp.tile([P, 2, N], bf16, tag="x", name=f"x{b}")
                    for ct in range(2):
                        nc.scalar.dma_start(out=x_sb[:, ct, :],
                                                              in_=xb[b, ct * P:(ct + 1) * P, :])
                qkv[b] = dict(
                    x=x_sb,
                    q=qk.tile([P, 2, N], bf16, tag="q", name=f"q{b}"),
                    k=qk.tile([P, 2, N], bf16, tag="k", name=f"k{b}"),
                    vt=vtp.tile([P, 8, 256], bf16, tag="vt", name=f"vt{b}"))
                r_tiles[b] = rp.tile([P, 2, N], bf16, tag="r", name=f"r{b}")
                # k(g0,nt0)+q(g0,nt0) suffice for the first four score
                # m-tiles (scores(mt) read k m-cols mt*128..+128 < 512);
                # k(g0,nt1) is only needed from mt4, so it goes last
                for kind, g, i in (("k", 0, 0), ("q", 0, 0), ("k", 0, 1)):
                    qkv_piece(b, kind, g, i)
                rest = [("q", 0, 1), ("q", 1, 0), ("k", 1, 0), ("k", 1, 1),
                        ("q", 1, 1)]
                vts = [("vt", 0, mt) for mt in range(8)]
                # b=0: its OWN AV waves interleave with its exp phase, so
                # vt pieces must enter the instruction stream first (wave
                # A(mt) reads vt(mt); a reader can only wait on writers
                # already issued). Later-b q/k pieces are needed sooner.
                rest = vts + rest if b == 0 else rest + vts
                return [lambda kind=kind, g=g, i=i: qkv_piece(b, kind, g, i)
                        for kind, g, i in rest]

            def avden_chunks(pend):
                """The pending iteration's AV + denominator matmuls as 16
                chunks: per m-tile one 4-wide AV wave then one 4-wide
                denominator wave. has_written clears are region-scoped
                (probe-verified): each head's mt==0 matmul starts its own
                accumulation."""
                b, g, nt = pend["key"]
                av, den, e_all = pend["av"], pend["den"], pend["e"]
                vt_sb = qkv[b]["vt"]
                chunks = []
                for mt in range(8):
                    def av_wave(mt=mt, g=g):
                        for p in range(4):
                            h = 4 * g + p
                            nc.tensor.matmul(
                                av[32 * p:32 * p + 32, :],
                                lhsT=vt_sb[:, mt, 32 * h:32 * h + 32],
                                rhs=e_all[:, mt, p * NT:(p + 1) * NT],
                                start=(mt == 0), stop=(mt == 7),
                                tile_position=(0, 32 * p),
                                skip_group_check=True)
                    def den_wave(mt=mt):
                        for p in range(4):
                            nc.tensor.matmul(
                                den[32 * p:32 * p + 32, :],
                                lhsT=ones_sb[:],
                                rhs=e_all[:, mt, p * NT:(p + 1) * NT],
                                start=(mt == 0), stop=(mt == 7),
                                tile_position=(0, 32 * p),
                                skip_group_check=True)
                    chunks.append(av_wave)
                    chunks.append(den_wave)
                return chunks

            def emit_proj(b, nt2s):
                """Output projection for batch b, restricted to the given
                n-halves (each half only needs the r columns of that half,
                letting the first half run before the final normalize)."""
                r_sb = r_tiles[b]
                for ct in range(2):
                    for nt2 in nt2s:
                        y_sb = yp.tile([P, NT], bf16, tag="y",
                                       name=f"y{b}{ct}{nt2}")
                        ps = rot_tile(f"proj_{b}{ct}{nt2}")
                        for gg in range(2):
                            nc.tensor.matmul(
                                ps[:],
                                lhsT=w_sb[:, gg, 768 + ct * P:768 + (ct + 1) * P],
                                rhs=r_sb[:, gg, nt2 * NT:(nt2 + 1) * NT],
                                start=(gg == 0), stop=(gg == 1))
                        nc.scalar.activation(
                            out=y_sb[:], in_=ps[:], func=Act.Identity,
                            bias=bias_ap("p", ct))
                        nc.sync.dma_start(
                            out=out[b, ct * P:(ct + 1) * P,
                                    nt2 * NT:(nt2 + 1) * NT],
                            in_=y_sb[:])

            def emit_finalize(pend):
                """normalize (relu-before-divide) for an iteration whose
                AV/den accumulation is fully issued."""
                b, g, nt = pend["key"]
                av, den = pend["av"], pend["den"]
                recip = rp.tile([P, NT], f32, tag="recip")
                nc.vector.reciprocal_approx_fast(out=recip[:], in_=den[:])
                nc.vector.scalar_tensor_tensor(
                    out=r_tiles[b][:, g, nt * NT:(nt + 1) * NT],
                    in0=av[:], scalar=0.0, in1=recip[:],
                    op0=Alu.max, op1=Alu.mult)

            # ---------- software-pipelined main loop ----------
            # iteration i: scores+exp for (b,g,nt), interleaved with the
            # PREVIOUS iteration's AV/denominator waves (keeps the PE dense
            # so HAM stays at full clock), then the previous normalize.
            iters = [(b, g, nt) for b in range(NB)
                     for g in range(2) for nt in range(2)]
            qkv_queue = []
            carry_fns = []
            for it, (b, g, nt) in enumerate(iters):
                is_first = it == 0
                if is_first:
                    qkv_queue.extend(emit_qkv_head(0))
                q_sb, k_sb = qkv[b]["q"], qkv[b]["k"]
                av = ps_av.tile([P, NT], f32, tag="av")
                den = ps_den.tile([P, NT], f32, tag="den")
                e_all = ep.tile([P, 8, 4 * NT], bf16, tag="e")
                own = dict(key=(b, g, nt), av=av, den=den, e=e_all)
                # lag-0 pipeline: this iteration's own AV/den waves trail its
                # exp by one m-tile (a wave may only be ISSUED once its
                # e-writers are already in the instruction stream, and runs
                # once the semaphores clear). The PE's wave+score load per
                # m-tile sits below the score->exp chain period, so waves
                # absorb PE slack instead of backlogging into the tail.
                chunks = avden_chunks(own)
                ci = 0
                for mt in range(8):
                    # 4-way row-group-packed score matmuls. j0/j1 share the
                    # ACT-owned sa tile; j2/j3 land in single-bank tiles from
                    # a 3-deep rotation so the next mt's scores never wait on
                    # this mt's (slower, DVE-queued) fast-exp consumer.
                    s1 = ps_s.tile([P, NT], f32, tag="sa0",
                                   name=f"s_{b}{g}{nt}{mt}a")
                    s2 = ps_s.tile([P, NT], f32, tag="sa1",
                                   name=f"s_{b}{g}{nt}{mt}b")
                    s3 = rot_tile(f"s_{b}{g}{nt}{mt}c")
                    s4 = rot_tile(f"s_{b}{g}{nt}{mt}d")
                    dsts = [s1[:], s2[:], s3[:], s4[:]]
                    for j in range(4):
                        row = 32 * j
                        nc.tensor.matmul(
                            dsts[j],
                            lhsT=k_sb[row:row + KD, g, mt * P:(mt + 1) * P],
                            rhs=q_sb[row:row + KD, g, nt * NT:(nt + 1) * NT],
                            start=True, stop=True,
                            tile_position=(row, 0))
                    # exp, split 50/50: heads 0/2 exact on ScalarE, heads
                    # 1/3 fast-exp on VectorE — both engines release their
                    # own score banks independently, halving the
                    # score->exp->score chain period
                    nc.scalar.activation(out=e_all[:, mt, 0:NT],
                                         in_=s1[:], func=Act.Exp)
                    nc.vector.tensor_scalar(
                        out=e_all[:, mt, NT:2 * NT].bitcast(i16),
                        in0=s2[:], scalar1=FE_SCALE, scalar2=FE_OFF,
                        op0=Alu.mult, op1=Alu.add)
                    nc.scalar.activation(out=e_all[:, mt, 2 * NT:3 * NT],
                                         in_=s3[:], func=Act.Exp)
                    nc.vector.tensor_scalar(
                        out=e_all[:, mt, 3 * NT:4 * NT].bitcast(i16),
                        in0=s4[:], scalar1=FE_SCALE, scalar2=FE_OFF,
                        op0=Alu.mult, op1=Alu.add)
                    # the previous iteration's deferred drain/finalize runs
                    # AFTER this iteration's first scores+exps are issued, so
                    # its late-gated waves never idle the ScalarE/VectorE
                    # across the iteration boundary
                    if mt < len(carry_fns):
                        carry_fns[mt]()
                        if mt == len(carry_fns) - 1:
                            carry_fns = []
                    # waves lag TWO m-tiles behind their exp: a wave needs
                    # ALL of its e columns (including the often-late DVE
                    # fast-exp ones), and a queued wave that still waits
                    # blocks the next scores behind it in the in-order PE
                    # queue even though those only need the early sa-exp.
                    while ci < min(len(chunks), 2 * (mt - 1)):
                        chunks[ci]()
                        ci += 1
                    if qkv_queue and (is_first or mt >= 1):
                        qkv_queue.pop(0)()
                    if is_first and qkv_queue:
                        qkv_queue.pop(0)()
                def it_drain(own=own, chunks=chunks, ci=ci):
                    for k in range(ci, len(chunks)):
                        chunks[k]()
                def it_fin(own=own):
                    emit_finalize(own)
                def it_proj(b=b, g=g, nt=nt):
                    # output projection halves as their r columns close:
                    # (g,nt)==(1,0) closes nt2=0, (1,1) closes nt2=1
                    if (g, nt) == (1, 0):
                        emit_proj(b, (0,))
                    elif (g, nt) == (1, 1):
                        emit_proj(b, (1,))
                if it == len(iters) - 1:
                    it_drain()
                    it_fin()
                    it_proj()
                else:
                    # deferred into the next iteration: waves at its first
                    # beat, normalize at the second, projection at the third
                    carry_fns = [it_drain, it_fin, it_proj]
                if is_first and NB > 1:
                    qkv_queue.extend(emit_qkv_head(1))

    if not nc.is_finalized():
        nc.finalize()
    return nc


def _prep_consts(wq, sq, bq, wk, sk, bk, wv, sv, bv, wp, sp, bp):
    """Host-side weight prep. Returns dict of per-core-identical arrays."""
    wq_s = (sq[:, None] * wq).astype(np.float32)
    wk_s = (sk[:, None] * wk).astype(np.float32)
    wv_s = (sv[:, None] * wv).astype(np.float32)
    wp_s = (sp[:, None] * wp).astype(np.float32)

    def pad_qk(w_s, bias):
        wT_pad = np.zeros((256, 256), np.float32)   # [c, gcol]
        b_pad = np.zeros(256, np.float32)
        for g in range(2):
            for j in range(4):
                h = 4 * g + j
                col = 128 * g + 32 * j
                wT_pad[:, col:col + KD] = w_s[KD * h:KD * (h + 1), :].T
                b_pad[col:col + KD] = bias[KD * h:KD * (h + 1)]
        return (wT_pad.reshape(2, P, 256).astype(BF16),
                b_pad.reshape(2, P, 1).astype(np.float32))

    wqT, bqp = pad_qk(wq_s, bq)
    wkT, bkp = pad_qk(wk_s, bk)
    wvT = wv_s.T.copy().reshape(2, P, 256).astype(BF16)   # [c, dh]
    wpT = wp_s.T.copy().reshape(2, P, 256).astype(BF16)   # [dh, c]
    wall = np.concatenate([wqT, wkT, wvT, wpT], axis=2)   # [2, 128, 1024]
    wall = np.ascontiguousarray(wall.transpose(1, 0, 2))  # [128, 2, 1024]
    bpp = bp.reshape(2, P).astype(np.float32)
    # combined bias tensor: [partition, group, {q,k,-,p}]
    biases = np.zeros((P, 2, 4), np.float32)
    for g in range(2):
        biases[:, g, 0] = bqp[g, :, 0]
        biases[:, g, 1] = bkp[g, :, 0]
        biases[:, g, 3] = bpp[g]
    bvb = np.broadcast_to(bv.astype(np.float32), (P, 2 * P)).copy()
    return dict(wall=wall, biases=biases, bvb=bvb)


def make_in_maps(inputs):
    x = np.ascontiguousarray(inputs["x"]).reshape(B, C, N).astype(BF16)
    consts = _prep_consts(*[np.asarray(inputs[k], np.float32) for k in
                            ["wq", "sq", "bq", "wk", "sk", "bk",
                             "wv", "sv", "bv", "wp", "sp", "bp"]])
    in_maps = []
    for core in range(NCORES):
        m = dict(consts)
        m["xb"] = np.ascontiguousarray(x[NB * core:NB * (core + 1)])
        in_maps.append(m)
    return in_maps


def gather_out(results):
    parts = [np.asarray(results[i]["out"]).astype(np.float32)
             for i in range(NCORES)]
    return np.concatenate(parts, axis=0).reshape(B, C, H, W)


def get_nc():
    if "nc" not in _CACHE:
        _CACHE["nc"] = _build_nc()
    return _CACHE["nc"]


def kernel(**inputs):
    import os
    os.environ.setdefault("BASS_NEVER_TRACE", "1")
    from concourse.bass_utils import run_bass_kernel_spmd
    nc = get_nc()
    in_maps = make_in_maps(inputs)
    res = run_bass_kernel_spmd(nc, in_maps, core_ids=list(range(NCORES)),
                               trace=False)
    return gather_out(res.results)


if __name__ == "__main__":
    nc = _build_nc()
    print("built ok")
